# revision 17
# baseline (speedup 1.0000x reference)
"""Trainium2 Bass kernel for CohereAttention (QK-LayerNorm + interleaved RoPE +
GQA sliding-window attention), sharded over 8 NeuronCores.

Sharding: tensor-parallel over Q heads (4 per core); with H//KVH == 4 each core
owns exactly one KV head. Attention outputs are AllGathered (bf16, pipelined
per head) and o_proj is column-parallel (512 output features per core), so no
all-reduce is needed.

Device-side layouts are transposed ([feature, token]) so every matmul contracts
over the partition axis at full PE rate:
  - QK-LayerNorm mean subtraction is folded into the projection weights on the
    host (subtract per-head column mean), leaving an RMS-style normalization.
  - RoPE rotate-half is a partition pair-swap (DVE stream_shuffle) with the sign
    folded into the sin table on the host; the 1/std scaling is applied last so
    the trig muls run off the critical path of the variance reduction.
  - Scores are computed transposed (S^T[j, q]) so the PV matmul needs no
    transposes; the QK matmul for chunk kk+2 is issued before the PV matmul of
    chunk kk so the PE never stalls on the exp and stays at the full p-state.
  - Softmax denominator comes from a ones-vector matmul and is applied at the
    attention-output drain via reciprocal_approx_fast + partition_broadcast.
  - Sliding-window/causal masks are fixed relative patterns; they are applied
    by multiplying the probs with precomputed 0/1 bf16 tiles on the DVE.
  - Large DMAs are chunked: a single DMA queue moves only ~22GB/s, so
    monolithic transfers serialize behind one queue while 16 sit idle.

Phases are serialized (no attn/proj interleave): the PE is the bottleneck
engine and is in-order, so interleaving phases only caused PSUM-bank handoff
stalls and Exp<->Sqrt activation-table thrash on the scalar engine.
"""

import sys

sys.path.insert(0, "/opt/trn_rl_repo")

import numpy as np
import ml_dtypes

import concourse.bass as bass
import concourse.mybir as mybir
import concourse.tile as tile
from concourse import bacc
from concourse.bass import ts, ds
from concourse.bass_utils import run_bass_kernel_spmd

B, S, H, KVH, D, HID = 2, 2048, 32, 8, 128, 4096
WINDOW = 512
EPS = 1e-5
SCALE = float(D) ** -0.5
NC = 8
HPC = H // NC              # q heads per core (4)
QW = HPC * D               # q width per core (512)
OW = HID // NC             # o_proj output width per core (512)
FCH = HID // 128           # contraction chunks (32)
TT = 512                   # projection token tile
QT = 256                   # attention query tile
NKC = (WINDOW + QT) // 128  # key chunks per query tile window (6)
PIPE = 2                   # attention QK-ahead pipeline depth

BF16 = mybir.dt.bfloat16
F32 = mybir.dt.float32
F32R = mybir.dt.float32r
npbf16 = ml_dtypes.bfloat16

SWAP32 = [i ^ 1 for i in range(32)]  # adjacent-pair partition swap

_CACHE = {}


def _mask_tiles():
    """0/1 bf16 [128, QT] mask tiles for the window/causal chunk edges.

    Chunk kk covers keys j = i0 - WINDOW + 128*kk + p for queries i = i0 + q;
    valid iff 0 <= i - j < WINDOW. The pattern depends only on kk, not on qt.
    """
    p = np.arange(128)[:, None]
    q = np.arange(QT)[None, :]
    masks = {}
    for kk in range(NKC):
        keep = np.ones((128, QT), dtype=bool)
        if kk * 128 < QT:               # window edge: p >= q + 1 - 128*kk
            keep &= p >= q + 1 - 128 * kk
        if kk * 128 > WINDOW - 128:     # causal edge: p <= q + WINDOW - 128*kk
            keep &= p <= q + WINDOW - 128 * kk
        if not keep.all():
            masks[kk] = keep.astype(npbf16)
    return masks


def _build_module():
    nc = bacc.Bacc(
        "TRN2",
        target_bir_lowering=False,
        debug=False,
        enable_asserts=False,
        num_devices=NC,
    )

    hT = nc.dram_tensor("hT", [B, HID, S], BF16, kind="ExternalInput").ap()
    cosT = nc.dram_tensor("cosT", [B, D, S], BF16, kind="ExternalInput").ap()
    sinT = nc.dram_tensor("sinT", [B, D, S], BF16, kind="ExternalInput").ap()
    wq = nc.dram_tensor("wq", [HID, QW], BF16, kind="ExternalInput").ap()
    wk = nc.dram_tensor("wk", [HID, D], BF16, kind="ExternalInput").ap()
    wv = nc.dram_tensor("wv", [HID, D], BF16, kind="ExternalInput").ap()
    wo = nc.dram_tensor("wo", [HID, OW], BF16, kind="ExternalInput").ap()
    winvq = nc.dram_tensor("winvq", [D, 1], BF16, kind="ExternalInput").ap()
    winvk = nc.dram_tensor("winvk", [D, 1], BF16, kind="ExternalInput").ap()
    out = nc.dram_tensor("out", [B * S, OW], BF16, kind="ExternalOutput").ap()

    agin = [
        nc.dram_tensor(f"agin{b}", [QW, S], BF16, kind="Internal").ap()
        for b in range(B)
    ]
    agout = [
        nc.dram_tensor(
            f"agout{b}", [HID, S], BF16, kind="Internal", addr_space="Shared"
        ).ap()
        for b in range(B)
    ]

    ident_d = nc.inline_tensor(np.eye(128, dtype=npbf16), name="ident").ap()
    ones_d = nc.inline_tensor(np.ones((128, 1), dtype=npbf16), name="onesv").ap()
    onesr_d = nc.inline_tensor(np.ones((1, 128), dtype=npbf16),
                               name="onesr").ap()
    np_masks = _mask_tiles()
    mask_d = {
        kk: nc.inline_tensor(m, name=f"mask{kk}").ap()
        for kk, m in np_masks.items()
    }

    rg = [list(range(NC))]

    with tile.TileContext(nc) as tc, \
            tc.tile_pool(name="sb", bufs=1) as sb, \
            tc.tile_pool(name="ps", bufs=1, space="PSUM") as ps:

        # --- resident weights / constants.  The first f-chunks of wq/wk/wv go
        # out first (each dma_start pays ~1us of DGE setup, so order by need);
        # masks/consts follow the weights since attention is ~250us away. ---
        wq_sb = sb.tile([128, FCH, QW], BF16, tag="wbig", bufs=1, name="wq_sb")
        wqv = wq.rearrange("(c p) n -> p c n", p=128)
        wk_sb = sb.tile([128, FCH, D], BF16, tag="wk", bufs=1, name="wk_sb")
        wkv = wk.rearrange("(c p) n -> p c n", p=128)
        wv_sb = sb.tile([128, FCH, D], BF16, tag="wv", bufs=1, name="wv_sb")
        wvv = wv.rearrange("(c p) n -> p c n", p=128)
        # only what the first few f-chunks of tt0 need; the rest of the
        # preload is interleaved into tt0's f-loop (hooks below) so the ht
        # stream isn't parked behind 12MB of weights in the DMA queues
        nc.sync.dma_start(wq_sb[:, 0:4, :], wqv[:, 0:4, :])
        nc.sync.dma_start(wk_sb[:, 0:8, :], wkv[:, 0:8, :])
        nc.sync.dma_start(wv_sb[:, 0:8, :], wvv[:, 0:8, :])
        for wc in range(1, 8):
            nc.sync.dma_start(wq_sb[:, ts(wc, 4), :], wqv[:, ts(wc, 4), :])
        nc.sync.dma_start(wk_sb[:, 8:32, :], wkv[:, 8:32, :])
        nc.sync.dma_start(wv_sb[:, 8:32, :], wvv[:, 8:32, :])
        winvq_sb = sb.tile([D, 1], BF16, tag="winvq", bufs=1, name="winvq_sb")
        nc.sync.dma_start(winvq_sb[:], winvq)
        winvk_sb = sb.tile([D, 1], BF16, tag="winvk", bufs=1, name="winvk_sb")
        nc.sync.dma_start(winvk_sb[:], winvk)
        onesr_sb = sb.tile([1, 128], BF16, tag="onesr", bufs=1, name="onesr_sb")
        nc.sync.dma_start(onesr_sb[:], onesr_d)
        eps_sb = sb.tile([1, 1], F32, tag="eps", bufs=1, name="eps_sb")
        nc.vector.memset(eps_sb[:], EPS)

        ident_sb = sb.tile([128, 128], BF16, tag="ident", bufs=1, name="ident_sb")
        ones_sb = sb.tile([128, 1], BF16, tag="ones", bufs=1, name="ones_sb")
        mask_sb = {}
        for kk in mask_d:
            mask_sb[kk] = sb.tile([128, QT], BF16, tag=f"mask{kk}", bufs=1,
                                  name=f"mask_sb{kk}")

        def _wload(fs):
            def go():
                lo = fs
                nc.sync.dma_start(wq_sb[:, ds(lo, 4), :], wqv[:, ds(lo, 4), :])
                if lo % 8 == 4:
                    nc.sync.dma_start(wk_sb[:, ds(lo + 4, 8), :],
                                      wkv[:, ds(lo + 4, 8), :])
                    nc.sync.dma_start(wv_sb[:, ds(lo + 4, 8), :],
                                      wvv[:, ds(lo + 4, 8), :])
            return go

        def _wconsts():
            nc.sync.dma_start(ident_sb[:], ident_d)
            nc.sync.dma_start(ones_sb[:], ones_d)
            for kk, md in mask_d.items():
                nc.sync.dma_start(mask_sb[kk][:], md)

        tt0_hooks = {4: _wload(4), 8: _wload(8), 12: _wload(12),
                     16: _wload(16), 20: _wload(20), 24: _wload(24),
                     28: _wload(28), 31: _wconsts}

        def ln_var(qps, winv_sb, cos_sb, sin_sb, tt):
            """Variance half of LayerNorm+RoPE: drains the psum accumulator,
            computes 1/std [1, TT], and runs the rstd-independent RoPE trig
            path. Returns state for ln_finish."""
            qsb = sb.tile([128, TT], BF16, tag="qsb", bufs=5, name="qsb")
            nc.scalar.copy(qsb[:], qps[:])  # sole reader: frees the psum bank
            sq = sb.tile([128, TT], BF16, tag="sq", bufs=3, name="sq")
            nc.vector.tensor_mul(sq[:], qsb[:], qsb[:])
            ssq = ps.tile([1, TT], F32, tag="misc", bufs=2, name="ssq")
            nc.tensor.matmul(ssq[:], winv_sb[:], sq[:], start=True, stop=True)
            std = sb.tile([1, TT], F32, tag="std", bufs=3, name="std")
            nc.scalar.activation(
                std[:], ssq[:], mybir.ActivationFunctionType.Sqrt,
                bias=eps_sb[:], scale=1.0 / D,
            )
            rinv = sb.tile([1, TT], F32, tag="rinv", bufs=3, name="rinv")
            nc.vector.reciprocal_approx_fast(rinv[:], std[:])
            rinvb = sb.tile([1, TT], BF16, tag="rinvb", bufs=3, name="rinvb")
            nc.vector.tensor_scalar_mul(rinvb[:], rinv[:], 1.0)
            qs = sb.tile([128, TT], BF16, tag="qs", bufs=3, name="qs")
            nc.vector.stream_shuffle(qs[:], qsb[:], SWAP32)
            m1 = sb.tile([128, TT], BF16, tag="t1", bufs=3, name="t1")
            nc.vector.tensor_mul(m1[:], qsb[:], cos_sb[:, ts(tt, TT)])
            m2 = sb.tile([128, TT], BF16, tag="t2", bufs=3, name="t2")
            nc.vector.tensor_mul(m2[:], qs[:], sin_sb[:, ts(tt, TT)])
            s12 = sb.tile([128, TT], BF16, tag="s12", bufs=3, name="s12")
            nc.vector.tensor_add(s12[:], m1[:], m2[:])
            return rinvb, s12

        def ln_finish(state, tt, dst):
            """Broadcast 1/std over partitions via a K=1 ones-row matmul on
            the PE (f32r: 1 cycle/row) and apply it."""
            rinv, s12 = state
            rbc = ps.tile([128, TT], F32, tag="misc", bufs=2, name="rbc")
            nc.tensor.matmul(rbc[:], onesr_sb[:], rinv[:],
                             start=True, stop=True)
            nc.vector.tensor_mul(dst[:, ts(tt, TT)], rbc[:], s12[:])

        qT = {}   # (b, h) -> [128, S] bf16 rope'd normalized q, transposed
        kT = {}   # b -> [128, S]
        Vn = {}   # b -> [128, S] (natural [j, d] in 128-col chunks)
        vT = {}   # b -> [128, S] transposed v (pre PE-transpose)
        trig = {}  # b -> (cos_sb, sin_sb)

        def proj_setup(b):
            cos_sb = sb.tile([128, S], BF16, tag="cos", bufs=1, name="cos_sb")
            sin_sb = sb.tile([128, S], BF16, tag="sin", bufs=1, name="sin_sb")
            for tc2 in range(2):
                nc.sync.dma_start(cos_sb[:, ts(tc2, S // 2)],
                                  cosT[b][:, ts(tc2, S // 2)])
                nc.sync.dma_start(sin_sb[:, ts(tc2, S // 2)],
                                  sinT[b][:, ts(tc2, S // 2)])
            trig[b] = (cos_sb, sin_sb)
            for h in range(HPC):
                qT[(b, h)] = sb.tile([128, S], BF16, tag="qT", bufs=8,
                                     name=f"qT{b}{h}")
            kT[b] = sb.tile([128, S], BF16, tag="kT", bufs=2, name=f"kT{b}")
            vT[b] = sb.tile([128, S], BF16, tag="vT", bufs=1, name=f"vT{b}")
            Vn[b] = sb.tile([128, S], BF16, tag="Vn", bufs=2, name=f"Vn{b}")

        def proj_tt(b, tt, hooks=None):
            with nc.named_scope(f"proj_b{b}"):
                cos_sb, sin_sb = trig[b]
                qps = [
                    ps.tile([128, TT], F32, tag=f"acc{i}", bufs=1,
                            name=f"qps{i}")
                    for i in range(HPC)
                ]
                kps = ps.tile([128, TT], F32, tag="acck", bufs=1, name="kps")
                vps = ps.tile([128, TT], F32, tag="accv", bufs=1, name="vps")
                for f in range(FCH):
                    if hooks and f in hooks:
                        hooks[f]()
                    ht_t = sb.tile([128, TT], BF16, tag="ht", bufs=7,
                                   name="ht_t")
                    nc.sync.dma_start(
                        ht_t[:], hT[b, ds(f * 128, 128), ts(tt, TT)]
                    )
                    st = f == 0
                    sp = f == FCH - 1
                    for h in range(HPC):
                        nc.tensor.matmul(
                            qps[h][:], wq_sb[:, f, ts(h, D)], ht_t[:],
                            start=st, stop=sp,
                        )
                    nc.tensor.matmul(kps[:], wk_sb[:, f, :], ht_t[:],
                                     start=st, stop=sp)
                    nc.tensor.matmul(vps[:], wv_sb[:, f, :], ht_t[:],
                                     start=st, stop=sp)
                states = []
                for h in range(HPC):
                    states.append(ln_var(qps[h], winvq_sb, cos_sb, sin_sb, tt))
                states.append(ln_var(kps, winvk_sb, cos_sb, sin_sb, tt))
                nc.scalar.copy(vT[b][:, ts(tt, TT)], vps[:])
                for h in range(HPC):
                    ln_finish(states[h], tt, qT[(b, h)])
                ln_finish(states[HPC], tt, kT[b])

        def proj_vtrans(b):
            # transpose v to natural [j, d] layout for the PV matmul
            with nc.named_scope(f"proj_b{b}"):
                for j in range(S // 128):
                    tp = ps.tile([128, 128], BF16, tag="misc", bufs=2, name="tp")
                    nc.tensor.transpose(tp[:], vT[b][:, ts(j, 128)], ident_sb[:])
                    nc.scalar.copy(Vn[b][:, ts(j, 128)], tp[:])

        def attn_head(b, h):
            """Transposed flash-style attention, software-pipelined two ways:
            the PV/denominator matmuls for chunk kk are issued after the QK
            matmul for chunk kk+PIPE (so the PE never waits on exp), and each
            qt's drain (1/l broadcast via K=1 PE matmul + final scale) is
            deferred into qt+1 (so the PE never waits on the reciprocal)."""
            with nc.named_scope(f"attn_b{b}"):
                attn_sb = sb.tile([128, S], BF16, tag="attn", bufs=2,
                                  name="attn_sb")
                dpend = []

                def drain_flush():
                    opsp, linvp, i0p = dpend.pop(0)
                    lbc = ps.tile([128, QT], F32, tag="misc", bufs=2,
                                  name="lbc")
                    nc.tensor.matmul(lbc[:], onesr_sb[:], linvp[:],
                                     start=True, stop=True)
                    # both operands cannot be PSUM (ISA s2s2d2)
                    nc.vector.tensor_mul(attn_sb[:, ds(i0p, QT)], lbc[:],
                                         opsp[:])

                for qt in range(S // QT):
                    i0 = qt * QT
                    kstart = max(0, (WINDOW - i0) // 128)
                    ops = ps.tile([128, QT], F32,
                                  tag="acc3" if qt % 2 == 0 else "acck",
                                  bufs=1, name="ops")
                    lps = ps.tile([1, QT], F32,
                                  tag="accv" if qt % 2 == 0 else "misc",
                                  bufs=1 if qt % 2 == 0 else 2, name="lps")
                    pend = []

                    def flush_one(stop):
                        ptp, j0p, firstp = pend.pop(0)
                        nc.tensor.matmul(
                            ops[:], Vn[b][:, ds(j0p, 128)], ptp[:],
                            start=firstp, stop=stop,
                        )
                        nc.tensor.matmul(
                            lps[:], ones_sb[:], ptp[:],
                            start=firstp, stop=stop,
                        )

                    for kk in range(kstart, NKC):
                        j0 = i0 - WINDOW + kk * 128
                        sps = ps.tile([128, QT], F32, tag=f"acc{kk % 3}",
                                      bufs=1, name="sps")
                        nc.tensor.matmul(
                            sps[:], kT[b][:, ds(j0, 128)],
                            qT[(b, h)][:, ds(i0, QT)],
                            start=True, stop=True,
                        )
                        if len(pend) == PIPE:
                            flush_one(False)
                        if kk == kstart + 1 and dpend:
                            drain_flush()
                        pt = sb.tile([128, QT], BF16, tag="pt", bufs=6,
                                     name="pt")
                        nc.scalar.activation(
                            pt[:], sps[:], mybir.ActivationFunctionType.Exp,
                            scale=SCALE,
                        )
                        if kk in mask_sb:
                            pm = sb.tile([128, QT], BF16, tag="pm", bufs=6,
                                         name="pm")
                            nc.vector.tensor_mul(pm[:], pt[:], mask_sb[kk][:])
                            pt = pm
                        pend.append((pt, j0, kk == kstart))
                    while pend:
                        flush_one(len(pend) == 1)
                    linv = sb.tile([1, QT], F32, tag="linv", bufs=3,
                                   name="linv")
                    nc.vector.reciprocal_approx_fast(linv[:], lps[:])
                    linvb = sb.tile([1, QT], BF16, tag="linvb", bufs=3,
                                    name="linvb")
                    nc.vector.tensor_scalar_mul(linvb[:], linv[:], 1.0)
                    opsb = sb.tile([128, QT], BF16, tag="opsb", bufs=3,
                                   name="opsb")
                    nc.vector.tensor_scalar_mul(opsb[:], ops[:], 1.0)
                    dpend.append((opsb, linvb, i0))
                while dpend:
                    drain_flush()
                # chunked so the AllGather isn't gated on one 20us DMA queue
                for ac in range(4):
                    nc.sync.dma_start(
                        agin[b][ts(h, 128), ts(ac, TT)],
                        attn_sb[:, ts(ac, TT)],
                    )

        def ag_head(b, h):
            # per-head AllGather: pipelines the collective behind the
            # remaining attention heads. agout feature order is
            # [head, core, d]; wo rows are permuted host-side to match.
            nc.gpsimd.collective_compute(
                "AllGather",
                mybir.AluOpType.bypass,
                replica_groups=rg,
                ins=[agin[b][ts(h, 128), :]],
                outs=[agout[b][ds(h * NC * D, NC * D), :]],
            )

        def oproj_load_w():
            wo_sb = _build_module.wo_sb = sb.tile(
                [128, FCH, OW], BF16, tag="wbig", bufs=1, name="wo_sb"
            )
            wov = wo.rearrange("(c p) n -> p c n", p=128)
            for wc in range(8):
                nc.sync.dma_start(wo_sb[:, ts(wc, 4), :], wov[:, ts(wc, 4), :])

        def oproj_phase(b, tqs=None):
            with nc.named_scope(f"oproj_b{b}"):
                wo_sb = _build_module.wo_sb
                agv = agout[b].rearrange("(c p) t -> p c t", p=128)
                for tq in (tqs if tqs is not None else range(S // 256)):
                    og = sb.tile([128, FCH, 256], BF16, tag="og", bufs=2,
                                 name="og")
                    # 8 slice loads: spreads queues and each depends only on
                    # its own per-head AllGather
                    for oc in range(8):
                        nc.sync.dma_start(
                            og[:, ts(oc, 4), :],
                            agv[:, ts(oc, 4), ts(tq, 256)],
                        )
                    for t2 in range(2):
                        po = ps.tile([128, OW], F32, tag="misc", bufs=2,
                                     name="po")
                        for c in range(FCH):
                            nc.tensor.matmul(
                                po[:], og[:, c, ts(t2, 128)], wo_sb[:, c, :],
                                start=(c == 0), stop=(c == FCH - 1),
                            )
                        ot = sb.tile([128, OW], BF16, tag="ot", bufs=2,
                                     name="ot")
                        nc.scalar.copy(ot[:], po[:])
                        nc.sync.dma_start(
                            out[ds(b * S + tq * 256 + t2 * 128, 128), :], ot[:]
                        )

        # Both projections run before any attention so that the AllGather
        # receive traffic (which self-times on cross-core semaphores and
        # parks descriptors in the shared DMA queues until the slowest core
        # produces the input) lands during the DMA-light attention phases
        # instead of starving the projection ht stream.
        proj_setup(0)
        tt0_hooks and None; _wconsts(); proj_tt(0, 0)
        for tt in range(1, S // TT):
            proj_tt(0, tt)
        proj_vtrans(0)
        proj_setup(1)
        for tt in range(S // TT):
            proj_tt(1, tt)
        oproj_load_w()
        proj_vtrans(1)
        for h in range(HPC):
            attn_head(0, h)
            ag_head(0, h)
        for h in range(HPC):
            attn_head(1, h)
            ag_head(1, h)
        oproj_phase(0)
        oproj_phase(1)

    nc.compile()
    return nc


def _prep_inputs(inputs):
    hidden = np.asarray(inputs["hidden_states"], np.float32)
    pos = np.asarray(inputs["position_ids"])
    cos = np.asarray(inputs["cos"], np.float32)
    sin = np.asarray(inputs["sin"], np.float32)
    wq = np.asarray(inputs["wq"], np.float32)
    wk = np.asarray(inputs["wk"], np.float32)
    wv = np.asarray(inputs["wv"], np.float32)
    wo = np.asarray(inputs["wo"], np.float32)
    qw = np.asarray(inputs["q_norm_w"], np.float32)
    kw = np.asarray(inputs["k_norm_w"], np.float32)

    hT = np.ascontiguousarray(hidden.transpose(0, 2, 1)).astype(npbf16)
    cosT = np.ascontiguousarray(cos[pos].transpose(0, 2, 1)).astype(npbf16)
    sinT_f = sin[pos].transpose(0, 2, 1).copy()
    sinT_f[:, 0::2, :] *= -1.0
    sinT = np.ascontiguousarray(sinT_f).astype(npbf16)

    winvq = (1.0 / np.where(qw == 0, 1, qw) ** 2).astype(npbf16).reshape(D, 1)
    winvk = (1.0 / np.where(kw == 0, 1, kw) ** 2).astype(npbf16).reshape(D, 1)

    # wo row permutation for per-head AllGather: agout feature order is
    # [h_local, core, d]; original rows are [core, h_local, d].
    hh = np.arange(HID)
    h_l = hh // (NC * D)
    core = (hh // D) % NC
    dd = hh % D
    wo_perm = wo[core * QW + h_l * D + dd, :]

    in_maps = []
    for c in range(NC):
        wq_c = wq[:, c * QW:(c + 1) * QW].copy()
        for j in range(HPC):
            blk = wq_c[:, j * D:(j + 1) * D]
            blk -= blk.mean(axis=1, keepdims=True)
            blk *= qw[None, :]
        wk_c = wk[:, c * D:(c + 1) * D].copy()
        wk_c -= wk_c.mean(axis=1, keepdims=True)
        wk_c *= kw[None, :]
        in_maps.append({
            "hT": hT,
            "cosT": cosT,
            "sinT": sinT,
            "wq": np.ascontiguousarray(wq_c).astype(npbf16),
            "wk": np.ascontiguousarray(wk_c).astype(npbf16),
            "wv": np.ascontiguousarray(wv[:, c * D:(c + 1) * D]).astype(npbf16),
            "wo": np.ascontiguousarray(wo_perm[:, c * OW:(c + 1) * OW]).astype(npbf16),
            "winvq": winvq,
            "winvk": winvk,
        })
    return in_maps


def _run(inputs, **kwargs):
    if "nc" not in _CACHE:
        _CACHE["nc"] = _build_module()
    nc = _CACHE["nc"]
    in_maps = _prep_inputs(inputs)
    res = run_bass_kernel_spmd(nc, in_maps, core_ids=list(range(NC)), **kwargs)
    shards = [
        np.asarray(res.results[c]["out"], dtype=np.float32).reshape(B, S, OW)
        for c in range(NC)
    ]
    return np.concatenate(shards, axis=-1), res


def kernel(**inputs) -> np.ndarray:
    out, _ = _run(inputs)
    return out


if __name__ == "__main__":
    import reference
    ins = {k: np.asarray(v) for k, v in reference.setup_inputs().items()}
    expected = np.asarray(reference.reference(**reference.setup_inputs()))
    actual = kernel(**ins)
    err = np.linalg.norm(actual - expected) / np.linalg.norm(expected)
    print("Relative error:", err)


# revision 18
# speedup vs baseline: 1.0510x; 1.0510x over previous
"""Trainium2 Bass kernel for CohereAttention (QK-LayerNorm + interleaved RoPE +
GQA sliding-window attention), sharded over 8 NeuronCores.

Sharding: tensor-parallel over Q heads (4 per core); with H//KVH == 4 each core
owns exactly one KV head. Attention outputs are AllGathered (bf16, pipelined
per head) and o_proj is column-parallel (512 output features per core), so no
all-reduce is needed.

Device-side layouts are transposed ([feature, token]) so every matmul contracts
over the partition axis at full PE rate:
  - QK-LayerNorm mean subtraction is folded into the projection weights on the
    host (subtract per-head column mean), leaving an RMS-style normalization.
  - RoPE rotate-half is a partition pair-swap (DVE stream_shuffle) with the sign
    folded into the sin table on the host; the 1/std scaling is applied last so
    the trig muls run off the critical path of the variance reduction.
  - Scores are computed transposed (S^T[j, q]) so the PV matmul needs no
    transposes; the QK matmul for chunk kk+2 is issued before the PV matmul of
    chunk kk so the PE never stalls on the exp and stays at the full p-state.
  - Softmax denominator comes from a ones-vector matmul and is applied at the
    attention-output drain via reciprocal_approx_fast + partition_broadcast.
  - Sliding-window/causal masks are fixed relative patterns; they are applied
    by multiplying the probs with precomputed 0/1 bf16 tiles on the DVE.
  - Large DMAs are chunked: a single DMA queue moves only ~22GB/s, so
    monolithic transfers serialize behind one queue while 16 sit idle.

Phases are serialized (no attn/proj interleave): the PE is the bottleneck
engine and is in-order, so interleaving phases only caused PSUM-bank handoff
stalls and Exp<->Sqrt activation-table thrash on the scalar engine.
"""

import sys

sys.path.insert(0, "/opt/trn_rl_repo")

import numpy as np
import ml_dtypes

import concourse.bass as bass
import concourse.mybir as mybir
import concourse.tile as tile
from concourse import bacc
from concourse.bass import ts, ds
from concourse.bass_utils import run_bass_kernel_spmd

B, S, H, KVH, D, HID = 2, 2048, 32, 8, 128, 4096
WINDOW = 512
EPS = 1e-5
SCALE = float(D) ** -0.5
NC = 8
HPC = H // NC              # q heads per core (4)
QW = HPC * D               # q width per core (512)
OW = HID // NC             # o_proj output width per core (512)
FCH = HID // 128           # contraction chunks (32)
TT = 512                   # projection token tile
QT = 256                   # attention query tile
NKC = (WINDOW + QT) // 128  # key chunks per query tile window (6)
PIPE = 2                   # attention QK-ahead pipeline depth

BF16 = mybir.dt.bfloat16
F32 = mybir.dt.float32
F32R = mybir.dt.float32r
npbf16 = ml_dtypes.bfloat16

SWAP32 = [i ^ 1 for i in range(32)]  # adjacent-pair partition swap

_CACHE = {}


def _mask_tiles():
    """0/1 bf16 [128, QT] mask tiles for the window/causal chunk edges.

    Chunk kk covers keys j = i0 - WINDOW + 128*kk + p for queries i = i0 + q;
    valid iff 0 <= i - j < WINDOW. The pattern depends only on kk, not on qt.
    """
    p = np.arange(128)[:, None]
    q = np.arange(QT)[None, :]
    masks = {}
    for kk in range(NKC):
        keep = np.ones((128, QT), dtype=bool)
        if kk * 128 < QT:               # window edge: p >= q + 1 - 128*kk
            keep &= p >= q + 1 - 128 * kk
        if kk * 128 > WINDOW - 128:     # causal edge: p <= q + WINDOW - 128*kk
            keep &= p <= q + WINDOW - 128 * kk
        if not keep.all():
            masks[kk] = keep.astype(npbf16)
    return masks


def _build_module():
    nc = bacc.Bacc(
        "TRN2",
        target_bir_lowering=False,
        debug=False,
        enable_asserts=False,
        num_devices=NC,
    )

    hT = nc.dram_tensor("hT", [B, HID, S], BF16, kind="ExternalInput").ap()
    cosT = nc.dram_tensor("cosT", [B, D, S], BF16, kind="ExternalInput").ap()
    sinT = nc.dram_tensor("sinT", [B, D, S], BF16, kind="ExternalInput").ap()
    wq = nc.dram_tensor("wq", [HID, QW], BF16, kind="ExternalInput").ap()
    wk = nc.dram_tensor("wk", [HID, D], BF16, kind="ExternalInput").ap()
    wv = nc.dram_tensor("wv", [HID, D], BF16, kind="ExternalInput").ap()
    wo = nc.dram_tensor("wo", [HID, OW], BF16, kind="ExternalInput").ap()
    winvq = nc.dram_tensor("winvq", [D, 1], BF16, kind="ExternalInput").ap()
    winvk = nc.dram_tensor("winvk", [D, 1], BF16, kind="ExternalInput").ap()
    out = nc.dram_tensor("out", [B * S, OW], BF16, kind="ExternalOutput").ap()

    agin = [
        nc.dram_tensor(f"agin{b}", [QW, S], BF16, kind="Internal").ap()
        for b in range(B)
    ]
    agout = [
        nc.dram_tensor(
            f"agout{b}", [HID, S], BF16, kind="Internal", addr_space="Shared"
        ).ap()
        for b in range(B)
    ]

    ident_d = nc.inline_tensor(np.eye(128, dtype=npbf16), name="ident").ap()
    ones_d = nc.inline_tensor(np.ones((128, 1), dtype=npbf16), name="onesv").ap()
    onesr_d = nc.inline_tensor(np.ones((1, 128), dtype=npbf16),
                               name="onesr").ap()
    np_masks = _mask_tiles()
    mask_d = {
        kk: nc.inline_tensor(m, name=f"mask{kk}").ap()
        for kk, m in np_masks.items()
    }

    rg = [list(range(NC))]

    with tile.TileContext(nc) as tc, \
            tc.tile_pool(name="sb", bufs=1) as sb, \
            tc.tile_pool(name="ps", bufs=1, space="PSUM") as ps:

        # --- resident weights / constants.  The first f-chunks of wq/wk/wv go
        # out first (each dma_start pays ~1us of DGE setup, so order by need);
        # masks/consts follow the weights since attention is ~250us away. ---
        wq_sb = sb.tile([128, FCH, QW], BF16, tag="wbig", bufs=1, name="wq_sb")
        wqv = wq.rearrange("(c p) n -> p c n", p=128)
        wk_sb = sb.tile([128, FCH, D], BF16, tag="wk", bufs=1, name="wk_sb")
        wkv = wk.rearrange("(c p) n -> p c n", p=128)
        wv_sb = sb.tile([128, FCH, D], BF16, tag="wv", bufs=1, name="wv_sb")
        wvv = wv.rearrange("(c p) n -> p c n", p=128)
        # only what the first few f-chunks of tt0 need; the rest of the
        # preload is interleaved into tt0's f-loop (hooks below) so the ht
        # stream isn't parked behind 12MB of weights in the DMA queues
        nc.sync.dma_start(wq_sb[:, 0:4, :], wqv[:, 0:4, :])
        nc.sync.dma_start(wk_sb[:, 0:8, :], wkv[:, 0:8, :])
        nc.sync.dma_start(wv_sb[:, 0:8, :], wvv[:, 0:8, :])
        winvq_sb = sb.tile([D, 1], BF16, tag="winvq", bufs=1, name="winvq_sb")
        nc.sync.dma_start(winvq_sb[:], winvq)
        winvk_sb = sb.tile([D, 1], BF16, tag="winvk", bufs=1, name="winvk_sb")
        nc.sync.dma_start(winvk_sb[:], winvk)
        onesr_sb = sb.tile([1, 128], BF16, tag="onesr", bufs=1, name="onesr_sb")
        nc.sync.dma_start(onesr_sb[:], onesr_d)
        eps_sb = sb.tile([1, 1], F32, tag="eps", bufs=1, name="eps_sb")
        nc.vector.memset(eps_sb[:], EPS)

        ident_sb = sb.tile([128, 128], BF16, tag="ident", bufs=1, name="ident_sb")
        ones_sb = sb.tile([128, 1], BF16, tag="ones", bufs=1, name="ones_sb")
        mask_sb = {}
        for kk in mask_d:
            mask_sb[kk] = sb.tile([128, QT], BF16, tag=f"mask{kk}", bufs=1,
                                  name=f"mask_sb{kk}")

        def _wload(fs):
            def go():
                lo = fs + 4
                nc.sync.dma_start(wq_sb[:, ds(lo, 4), :], wqv[:, ds(lo, 4), :])
                if lo % 8 == 0:
                    nc.sync.dma_start(wk_sb[:, ds(lo, 8), :],
                                      wkv[:, ds(lo, 8), :])
                    nc.sync.dma_start(wv_sb[:, ds(lo, 8), :],
                                      wvv[:, ds(lo, 8), :])
            return go

        def _wconsts():
            nc.sync.dma_start(ident_sb[:], ident_d)
            nc.sync.dma_start(ones_sb[:], ones_d)
            for kk, md in mask_d.items():
                nc.sync.dma_start(mask_sb[kk][:], md)

        tt0_hooks = {0: _wload(0), 4: _wload(4), 8: _wload(8),
                     12: _wload(12), 16: _wload(16), 20: _wload(20),
                     24: _wload(24)}

        def ln_var(qps, winv_sb, cos_sb, sin_sb, tt):
            """Variance half of LayerNorm+RoPE: drains the psum accumulator,
            computes 1/std [1, TT], and runs the rstd-independent RoPE trig
            path. Returns state for ln_finish."""
            qsb = sb.tile([128, TT], BF16, tag="qsb", bufs=5, name="qsb")
            nc.scalar.copy(qsb[:], qps[:])  # sole reader: frees the psum bank
            sq = sb.tile([128, TT], BF16, tag="sq", bufs=3, name="sq")
            nc.vector.tensor_mul(sq[:], qsb[:], qsb[:])
            ssq = ps.tile([1, TT], F32, tag="misc", bufs=2, name="ssq")
            nc.tensor.matmul(ssq[:], winv_sb[:], sq[:], start=True, stop=True)
            std = sb.tile([1, TT], F32, tag="std", bufs=3, name="std")
            nc.scalar.activation(
                std[:], ssq[:], mybir.ActivationFunctionType.Sqrt,
                bias=eps_sb[:], scale=1.0 / D,
            )
            rinv = sb.tile([1, TT], F32, tag="rinv", bufs=3, name="rinv")
            nc.vector.reciprocal_approx_fast(rinv[:], std[:])
            rinvb = sb.tile([1, TT], BF16, tag="rinvb", bufs=3, name="rinvb")
            nc.vector.tensor_scalar_mul(rinvb[:], rinv[:], 1.0)
            qs = sb.tile([128, TT], BF16, tag="qs", bufs=3, name="qs")
            nc.vector.stream_shuffle(qs[:], qsb[:], SWAP32)
            m1 = sb.tile([128, TT], BF16, tag="t1", bufs=3, name="t1")
            nc.vector.tensor_mul(m1[:], qsb[:], cos_sb[:, ts(tt, TT)])
            m2 = sb.tile([128, TT], BF16, tag="t2", bufs=3, name="t2")
            nc.vector.tensor_mul(m2[:], qs[:], sin_sb[:, ts(tt, TT)])
            s12 = sb.tile([128, TT], BF16, tag="s12", bufs=3, name="s12")
            nc.vector.tensor_add(s12[:], m1[:], m2[:])
            return rinvb, s12

        def ln_finish(state, tt, dst):
            """Broadcast 1/std over partitions via a K=1 ones-row matmul on
            the PE (f32r: 1 cycle/row) and apply it."""
            rinv, s12 = state
            rbc = ps.tile([128, TT], F32, tag="misc", bufs=2, name="rbc")
            nc.tensor.matmul(rbc[:], onesr_sb[:], rinv[:],
                             start=True, stop=True)
            nc.vector.tensor_mul(dst[:, ts(tt, TT)], rbc[:], s12[:])

        qT = {}   # (b, h) -> [128, S] bf16 rope'd normalized q, transposed
        kT = {}   # b -> [128, S]
        Vn = {}   # b -> [128, S] (natural [j, d] in 128-col chunks)
        vT = {}   # b -> [128, S] transposed v (pre PE-transpose)
        trig = {}  # b -> (cos_sb, sin_sb)

        def proj_setup(b):
            cos_sb = sb.tile([128, S], BF16, tag="cos", bufs=1, name="cos_sb")
            sin_sb = sb.tile([128, S], BF16, tag="sin", bufs=1, name="sin_sb")
            for tc2 in range(2):
                nc.sync.dma_start(cos_sb[:, ts(tc2, S // 2)],
                                  cosT[b][:, ts(tc2, S // 2)])
                nc.sync.dma_start(sin_sb[:, ts(tc2, S // 2)],
                                  sinT[b][:, ts(tc2, S // 2)])
            trig[b] = (cos_sb, sin_sb)
            for h in range(HPC):
                qT[(b, h)] = sb.tile([128, S], BF16, tag="qT", bufs=8,
                                     name=f"qT{b}{h}")
            kT[b] = sb.tile([128, S], BF16, tag="kT", bufs=2, name=f"kT{b}")
            vT[b] = sb.tile([128, S], BF16, tag="vT", bufs=1, name=f"vT{b}")
            Vn[b] = sb.tile([128, S], BF16, tag="Vn", bufs=2, name=f"Vn{b}")

        def proj_tt(b, tt, hooks=None):
            with nc.named_scope(f"proj_b{b}"):
                cos_sb, sin_sb = trig[b]
                qps = [
                    ps.tile([128, TT], F32, tag=f"acc{i}", bufs=1,
                            name=f"qps{i}")
                    for i in range(HPC)
                ]
                kps = ps.tile([128, TT], F32, tag="acck", bufs=1, name="kps")
                vps = ps.tile([128, TT], F32, tag="accv", bufs=1, name="vps")
                for f in range(FCH):
                    if hooks and f in hooks:
                        hooks[f]()
                    ht_t = sb.tile([128, TT], BF16, tag="ht", bufs=7,
                                   name="ht_t")
                    nc.sync.dma_start(
                        ht_t[:], hT[b, ds(f * 128, 128), ts(tt, TT)]
                    )
                    st = f == 0
                    sp = f == FCH - 1
                    for h in range(HPC):
                        nc.tensor.matmul(
                            qps[h][:], wq_sb[:, f, ts(h, D)], ht_t[:],
                            start=st, stop=sp,
                        )
                    nc.tensor.matmul(kps[:], wk_sb[:, f, :], ht_t[:],
                                     start=st, stop=sp)
                    nc.tensor.matmul(vps[:], wv_sb[:, f, :], ht_t[:],
                                     start=st, stop=sp)
                states = []
                for h in range(HPC):
                    states.append(ln_var(qps[h], winvq_sb, cos_sb, sin_sb, tt))
                states.append(ln_var(kps, winvk_sb, cos_sb, sin_sb, tt))
                nc.scalar.copy(vT[b][:, ts(tt, TT)], vps[:])
                for h in range(HPC):
                    ln_finish(states[h], tt, qT[(b, h)])
                ln_finish(states[HPC], tt, kT[b])

        def proj_vtrans(b):
            # transpose v to natural [j, d] layout for the PV matmul
            with nc.named_scope(f"proj_b{b}"):
                for j in range(S // 128):
                    tp = ps.tile([128, 128], BF16, tag="misc", bufs=2, name="tp")
                    nc.tensor.transpose(tp[:], vT[b][:, ts(j, 128)], ident_sb[:])
                    nc.scalar.copy(Vn[b][:, ts(j, 128)], tp[:])

        def attn_head(b, h):
            """Transposed flash-style attention, software-pipelined two ways:
            the PV/denominator matmuls for chunk kk are issued after the QK
            matmul for chunk kk+PIPE (so the PE never waits on exp), and each
            qt's drain (1/l broadcast via K=1 PE matmul + final scale) is
            deferred into qt+1 (so the PE never waits on the reciprocal)."""
            with nc.named_scope(f"attn_b{b}"):
                attn_sb = sb.tile([128, S], BF16, tag="attn", bufs=2,
                                  name="attn_sb")
                dpend = []

                def drain_flush():
                    opsp, linvp, i0p = dpend.pop(0)
                    lbc = ps.tile([128, QT], F32, tag="misc", bufs=2,
                                  name="lbc")
                    nc.tensor.matmul(lbc[:], onesr_sb[:], linvp[:],
                                     start=True, stop=True)
                    # both operands cannot be PSUM (ISA s2s2d2)
                    nc.vector.tensor_mul(attn_sb[:, ds(i0p, QT)], lbc[:],
                                         opsp[:])

                for qt in range(S // QT):
                    i0 = qt * QT
                    kstart = max(0, (WINDOW - i0) // 128)
                    ops = ps.tile([128, QT], F32,
                                  tag="acc3" if qt % 2 == 0 else "acck",
                                  bufs=1, name="ops")
                    lps = ps.tile([1, QT], F32,
                                  tag="accv" if qt % 2 == 0 else "misc",
                                  bufs=1 if qt % 2 == 0 else 2, name="lps")
                    pend = []

                    def flush_one(stop):
                        ptp, j0p, firstp = pend.pop(0)
                        nc.tensor.matmul(
                            ops[:], Vn[b][:, ds(j0p, 128)], ptp[:],
                            start=firstp, stop=stop,
                        )
                        nc.tensor.matmul(
                            lps[:], ones_sb[:], ptp[:],
                            start=firstp, stop=stop,
                        )

                    for kk in range(kstart, NKC):
                        j0 = i0 - WINDOW + kk * 128
                        sps = ps.tile([128, QT], F32, tag=f"acc{kk % 3}",
                                      bufs=1, name="sps")
                        nc.tensor.matmul(
                            sps[:], kT[b][:, ds(j0, 128)],
                            qT[(b, h)][:, ds(i0, QT)],
                            start=True, stop=True,
                        )
                        if len(pend) == PIPE:
                            flush_one(False)
                        if kk == kstart + 1 and dpend:
                            drain_flush()
                        pt = sb.tile([128, QT], BF16, tag="pt", bufs=6,
                                     name="pt")
                        nc.scalar.activation(
                            pt[:], sps[:], mybir.ActivationFunctionType.Exp,
                            scale=SCALE,
                        )
                        if kk in mask_sb:
                            pm = sb.tile([128, QT], BF16, tag="pm", bufs=6,
                                         name="pm")
                            nc.vector.tensor_mul(pm[:], pt[:], mask_sb[kk][:])
                            pt = pm
                        pend.append((pt, j0, kk == kstart))
                    while pend:
                        flush_one(len(pend) == 1)
                    linv = sb.tile([1, QT], F32, tag="linv", bufs=3,
                                   name="linv")
                    nc.vector.reciprocal_approx_fast(linv[:], lps[:])
                    linvb = sb.tile([1, QT], BF16, tag="linvb", bufs=3,
                                    name="linvb")
                    nc.vector.tensor_scalar_mul(linvb[:], linv[:], 1.0)
                    opsb = sb.tile([128, QT], BF16, tag="opsb", bufs=3,
                                   name="opsb")
                    nc.vector.tensor_scalar_mul(opsb[:], ops[:], 1.0)
                    dpend.append((opsb, linvb, i0))
                while dpend:
                    drain_flush()
                # chunked so the AllGather isn't gated on one 20us DMA queue
                for ac in range(4):
                    nc.sync.dma_start(
                        agin[b][ts(h, 128), ts(ac, TT)],
                        attn_sb[:, ts(ac, TT)],
                    )

        def ag_head(b, h):
            # per-head AllGather: pipelines the collective behind the
            # remaining attention heads. agout feature order is
            # [head, core, d]; wo rows are permuted host-side to match.
            nc.gpsimd.collective_compute(
                "AllGather",
                mybir.AluOpType.bypass,
                replica_groups=rg,
                ins=[agin[b][ts(h, 128), :]],
                outs=[agout[b][ds(h * NC * D, NC * D), :]],
            )

        def oproj_load_w():
            wo_sb = _build_module.wo_sb = sb.tile(
                [128, FCH, OW], BF16, tag="wbig", bufs=1, name="wo_sb"
            )
            wov = wo.rearrange("(c p) n -> p c n", p=128)
            for wc in range(8):
                nc.sync.dma_start(wo_sb[:, ts(wc, 4), :], wov[:, ts(wc, 4), :])

        def oproj_phase(b, tqs=None):
            with nc.named_scope(f"oproj_b{b}"):
                wo_sb = _build_module.wo_sb
                agv = agout[b].rearrange("(c p) t -> p c t", p=128)
                for tq in (tqs if tqs is not None else range(S // 256)):
                    og = sb.tile([128, FCH, 256], BF16, tag="og", bufs=2,
                                 name="og")
                    # 8 slice loads: spreads queues and each depends only on
                    # its own per-head AllGather
                    for oc in range(8):
                        nc.sync.dma_start(
                            og[:, ts(oc, 4), :],
                            agv[:, ts(oc, 4), ts(tq, 256)],
                        )
                    for t2 in range(2):
                        po = ps.tile([128, OW], F32, tag="misc", bufs=2,
                                     name="po")
                        for c in range(FCH):
                            nc.tensor.matmul(
                                po[:], og[:, c, ts(t2, 128)], wo_sb[:, c, :],
                                start=(c == 0), stop=(c == FCH - 1),
                            )
                        ot = sb.tile([128, OW], BF16, tag="ot", bufs=2,
                                     name="ot")
                        nc.scalar.copy(ot[:], po[:])
                        nc.sync.dma_start(
                            out[ds(b * S + tq * 256 + t2 * 128, 128), :], ot[:]
                        )

        # Both projections run before any attention so that the AllGather
        # receive traffic (which self-times on cross-core semaphores and
        # parks descriptors in the shared DMA queues until the slowest core
        # produces the input) lands during the DMA-light attention phases
        # instead of starving the projection ht stream.
        proj_setup(0)
        proj_tt(0, 0, hooks=tt0_hooks)
        _wconsts()
        for tt in range(1, S // TT):
            proj_tt(0, tt)
        proj_vtrans(0)
        proj_setup(1)
        for tt in range(S // TT):
            proj_tt(1, tt)
        oproj_load_w()
        proj_vtrans(1)
        for h in range(HPC):
            attn_head(0, h)
            ag_head(0, h)
        for h in range(HPC):
            attn_head(1, h)
            ag_head(1, h)
        oproj_phase(0)
        oproj_phase(1)

    nc.compile()
    return nc


def _prep_inputs(inputs):
    hidden = np.asarray(inputs["hidden_states"], np.float32)
    pos = np.asarray(inputs["position_ids"])
    cos = np.asarray(inputs["cos"], np.float32)
    sin = np.asarray(inputs["sin"], np.float32)
    wq = np.asarray(inputs["wq"], np.float32)
    wk = np.asarray(inputs["wk"], np.float32)
    wv = np.asarray(inputs["wv"], np.float32)
    wo = np.asarray(inputs["wo"], np.float32)
    qw = np.asarray(inputs["q_norm_w"], np.float32)
    kw = np.asarray(inputs["k_norm_w"], np.float32)

    hT = np.ascontiguousarray(hidden.transpose(0, 2, 1)).astype(npbf16)
    cosT = np.ascontiguousarray(cos[pos].transpose(0, 2, 1)).astype(npbf16)
    sinT_f = sin[pos].transpose(0, 2, 1).copy()
    sinT_f[:, 0::2, :] *= -1.0
    sinT = np.ascontiguousarray(sinT_f).astype(npbf16)

    winvq = (1.0 / np.where(qw == 0, 1, qw) ** 2).astype(npbf16).reshape(D, 1)
    winvk = (1.0 / np.where(kw == 0, 1, kw) ** 2).astype(npbf16).reshape(D, 1)

    # wo row permutation for per-head AllGather: agout feature order is
    # [h_local, core, d]; original rows are [core, h_local, d].
    hh = np.arange(HID)
    h_l = hh // (NC * D)
    core = (hh // D) % NC
    dd = hh % D
    wo_perm = wo[core * QW + h_l * D + dd, :]

    in_maps = []
    for c in range(NC):
        wq_c = wq[:, c * QW:(c + 1) * QW].copy()
        for j in range(HPC):
            blk = wq_c[:, j * D:(j + 1) * D]
            blk -= blk.mean(axis=1, keepdims=True)
            blk *= qw[None, :]
        wk_c = wk[:, c * D:(c + 1) * D].copy()
        wk_c -= wk_c.mean(axis=1, keepdims=True)
        wk_c *= kw[None, :]
        in_maps.append({
            "hT": hT,
            "cosT": cosT,
            "sinT": sinT,
            "wq": np.ascontiguousarray(wq_c).astype(npbf16),
            "wk": np.ascontiguousarray(wk_c).astype(npbf16),
            "wv": np.ascontiguousarray(wv[:, c * D:(c + 1) * D]).astype(npbf16),
            "wo": np.ascontiguousarray(wo_perm[:, c * OW:(c + 1) * OW]).astype(npbf16),
            "winvq": winvq,
            "winvk": winvk,
        })
    return in_maps


def _run(inputs, **kwargs):
    if "nc" not in _CACHE:
        _CACHE["nc"] = _build_module()
    nc = _CACHE["nc"]
    in_maps = _prep_inputs(inputs)
    res = run_bass_kernel_spmd(nc, in_maps, core_ids=list(range(NC)), **kwargs)
    shards = [
        np.asarray(res.results[c]["out"], dtype=np.float32).reshape(B, S, OW)
        for c in range(NC)
    ]
    return np.concatenate(shards, axis=-1), res


def kernel(**inputs) -> np.ndarray:
    out, _ = _run(inputs)
    return out


if __name__ == "__main__":
    import reference
    ins = {k: np.asarray(v) for k, v in reference.setup_inputs().items()}
    expected = np.asarray(reference.reference(**reference.setup_inputs()))
    actual = kernel(**ins)
    err = np.linalg.norm(actual - expected) / np.linalg.norm(expected)
    print("Relative error:", err)
